# revision 1
# baseline (speedup 1.0000x reference)
"""Trainium2 Bass kernel for the Lineq2v2nano equivariant 2->2 layer.

Math (per sample b):
  out[i,j,f] = relu( x[i,j,:]@W0                                  (op0)
                   + totsum@W1' + bias                            (op1, const over i,j)
                   + rowsum[i]@W2'                                (op2, bcast over j)
                   + rowsum[j]@W3'                                (op3, bcast over i)
                   + delta_ij * (rowsum[i]@W4' + totsum@W5' + diag_bias) )

Kernel strategy (data-parallel, 4 samples per core on 8 cores):
  - load x[b] as the natural [128 i, 2048 (j,l)] tile (contiguous DMA)
  - PE-transpose 128-wide chunks -> xT[(j8,l), i] tiles
  - main term: per 512-wide psum slab, 2 matmuls with a block-diagonal
    W0 (K=(j8,l)=128, N=256) in float32r
  - rowsum via DVE binary add tree; transposed on PE
  - op2 via a K=16 matmul with W2 tiled across j (bcast over j)
  - op1/op3/bias collapse into a per-j [128,32] "column bias" computed by
    a tiny matmul, flattened to one partition by an SBUF->SBUF DMA and
    added to every i by a K=1 ones-matmul
  - relu on ACT during psum->SBUF eviction, store [128, 4096] per sample
  - the diagonal term is handled by computing relu'd diagonal rows
    separately ([128, 32] per sample) and overwriting out[b,i,i,:] with a
    strided-DRAM-AP store ordered after the main stores
"""

import os
import sys

sys.path.insert(0, "/opt/trn_rl_repo")

import numpy as np

N_CORES = 8
B, N, L, F = 32, 128, 16, 32
NAVG = 50.0
B_LOC = B // N_CORES  # samples per core

_CACHE = {}

LAST_EXEC_NS = None
LAST_RESULTS = None

def _build_module():
    import concourse.bass as bass
    import concourse.mybir as mybir
    from concourse import bacc
    from concourse.tile import TileContext, add_dep_helper

    f32 = mybir.dt.float32
    bf16 = mybir.dt.bfloat16
    JL = N * L      # 2048
    JF = N * F      # 4096

    nc = bacc.Bacc(None, target_bir_lowering=False)
    CP = 256 + 128 + 64 + 64 + 64 + 64 + JF  # wblk|ident|w34|wtot|w0d|bcat|w2t
    f32r = mybir.dt.float32r
    x_h = nc.declare_dram_parameter("x", [B_LOC, N, JL], f32r, isOutput=False)
    cpack_h = nc.declare_dram_parameter("cpack", [128, CP], bf16, isOutput=False)
    identr_h = nc.declare_dram_parameter("identr", [128, 128], f32r, isOutput=False)
    out_h = nc.declare_dram_parameter("out", [B_LOC, N, JF], f32, isOutput=True)

    from contextlib import ExitStack

    with TileContext(nc) as tc, ExitStack() as stack:
        consts = stack.enter_context(tc.tile_pool(name="consts", bufs=1))
        # single packed const load -> one DVE launder copy; everything PE
        # reads is a slice of cl (DVE-produced, keeps PE waits simple)
        cp0 = consts.tile([128, CP], bf16)
        cl = consts.tile([128, CP], bf16)
        # [W2-tiled ; colflat] combined moving operand, double-buffered by
        # sample parity (row 16 is rewritten per sample by the cf DMA)
        w2cf0 = consts.tile([17, JF], bf16)
        w2cf1 = consts.tile([17, JF], bf16)
        ones = consts.tile([1, 128], bf16)
        xdg_f = consts.tile([128, B_LOC * 16], f32r)
        xdg = consts.tile([128, B_LOC * 16], bf16)
        zdall = consts.tile([128, B_LOC * 32], f32)   # relu'd diagonal rows

        identr0 = consts.tile([128, 128], f32r)
        identr = consts.tile([128, 128], f32r)
        cp_dma = nc.sync.dma_start(out=cp0[:], in_=cpack_h[:])
        nc.sync.dma_start(out=identr0[:], in_=identr_h[:])
        nc.vector.memset(ones[:], 1.0)
        nc.vector.tensor_copy(identr[:], identr0[:])
        nc.vector.tensor_copy(cl[:], cp0[:])
        o_wblk, o_id, o_w34, o_wtot, o_w0d, o_bcat, o_w2t = (
            0, 256, 384, 448, 512, 576, 640)
        wblk = cl[:, o_wblk : o_wblk + 256]
        ident = cl[:, o_id : o_id + 128]
        w34 = cl[0:16, o_w34 : o_w34 + 64]
        wtot = cl[0:16, o_wtot : o_wtot + 64]
        w0d = cl[0:16, o_w0d : o_w0d + 64]
        bcat = cl[0:1, o_bcat : o_bcat + 64]
        nc.vector.tensor_copy(w2cf0[0:16, :], cl[0:16, o_w2t : o_w2t + JF])
        nc.vector.tensor_copy(w2cf1[0:16, :], cl[0:16, o_w2t : o_w2t + JF])
        # diagonal gather: x[b, i, i, :] -> [128 i, (b, l)] then cast
        x0 = x_h[:]
        xdiag_src = bass.AP(
            tensor=x0.tensor,
            offset=x0.offset,
            ap=[[N * L + L, 128], [N * JL, B_LOC], [1, L]],
        )
        nc.sync.dma_start(out=xdg_f[:], in_=xdiag_src)
        nc.vector.tensor_copy(xdg[:], xdg_f[:].bitcast(f32))

        xt_p = stack.enter_context(tc.tile_pool(name="xt", bufs=3))
        xts_p = stack.enter_context(tc.tile_pool(name="xts", bufs=2))
        osb_p = stack.enter_context(tc.tile_pool(name="osb", bufs=2))
        sm_p = stack.enter_context(tc.tile_pool(name="small", bufs=4))
        ps_t = stack.enter_context(tc.tile_pool(name="ps_t", bufs=2, space="PSUM"))
        ps_o = stack.enter_context(tc.tile_pool(name="ps_o", bufs=4, space="PSUM"))
        ps_s = stack.enter_context(tc.tile_pool(name="ps_s", bufs=2, space="PSUM"))

        store_insts = []
        for b in range(B_LOC):
            # ---- plain f32 load on the SP HWDGE ring: ring FIFO means
            # sample b's data always lands before sample b+1's ----
            xt = xt_p.tile([128, JL], f32r, tag="xt")
            xb = x_h[b]
            half = JL // 2
            nc.sync.dma_start(out=xt[:, 0:half], in_=xb[:, 0:half])
            nc.sync.dma_start(out=xt[:, half:JL], in_=xb[:, half:JL])

            # ---- rowsum (sum over j) via two half-trees + merge, so the
            # first half starts as soon as its load lands ----
            tr = sm_p.tile([128, 1024], bf16, tag="tree")
            xtf = xt[:].bitcast(f32)
            for hh in range(2):
                base = hh * 512
                nc.vector.tensor_add(
                    tr[:, base : base + 512],
                    xtf[:, 2 * base : 2 * base + 512],
                    xtf[:, 2 * base + 512 : 2 * base + 1024],
                )
                w = 256
                while w >= 16:
                    nc.vector.tensor_add(
                        tr[:, base : base + w],
                        tr[:, base : base + w],
                        tr[:, base + w : base + 2 * w],
                    )
                    w //= 2
            nc.vector.tensor_add(tr[:, 0:16], tr[:, 0:16], tr[:, 512:528])
            # raw rowsum (no /NAVG; folded into weights) sits in tr[:, 0:16]

            # ---- transpose [rowsum | ones-col] -> rstcat [17, 128] ----
            nc.vector.memset(tr[:, 16:17], 1.0)
            pt_rs = ps_s.tile([17, 128], bf16, tag="ps_small")
            nc.tensor.transpose(pt_rs[:], tr[:, 0:17], ident)
            rstcat = sm_p.tile([17, 128], bf16, tag="rst")
            nc.vector.tensor_copy(rstcat[:], pt_rs[:])
            rst = rstcat[0:16, :]

            # ---- totsum + tiny matmuls ----
            totc = sm_p.tile([16, 1], bf16, tag="totc")
            with nc.allow_low_precision(reason="totsum terms are tiny"):
                nc.vector.tensor_reduce(
                    out=totc[:], in_=pt_rs[0:16, :], axis=mybir.AxisListType.X,
                    op=mybir.AluOpType.add,
                )
            ptv = ps_s.tile([1, 64], f32, tag="ps_small")
            nc.tensor.matmul(ptv[:], lhsT=totc[:], rhs=wtot, start=True, stop=True)
            tv = sm_p.tile([1, 64], bf16, tag="tv")
            nc.vector.tensor_add(tv[:], ptv[:], bcat)
            tvs = sm_p.tile([1, 32], bf16, tag="tvs")
            nc.vector.tensor_add(tvs[:], tv[0:1, 0:32], tv[0:1, 32:64])

            # cd = [colbias2 | d]: rowsum@[W3p|W4p] + ones x tv
            pcd = ps_s.tile([128, 64], f32, tag="ps_small")
            nc.tensor.matmul(pcd[:], lhsT=rst, rhs=w34, start=True, stop=False)
            nc.tensor.matmul(pcd[:], lhsT=ones[:], rhs=tv[:], start=False, stop=True)
            cd = sm_p.tile([128, 64], bf16, tag="cd")
            nc.vector.tensor_copy(cd[:], pcd[:])

            # flatten colbias2 [128, 32] -> row 16 of this sample's w2cf
            w2cf = w2cf0 if b % 2 == 0 else w2cf1
            cf_dma = nc.sync.dma_start(out=w2cf[16:17, :], in_=cd[:, 0:32])

            # ---- transposes of x chunks: xts[(j8,l), i] ----
            # 4 transposes land in one [128, 512] psum bank -> 1 DVE copy
            xts = xts_p.tile([128, JL], bf16, tag="xts")
            for q in range(4):
                pt = ps_t.tile([128, 512], f32r, tag="pt")
                for k in range(4):
                    jb = 4 * q + k
                    nc.tensor.transpose(
                        pt[:, k * 128 : (k + 1) * 128],
                        xt[:, jb * 128 : (jb + 1) * 128], identr[:],
                    )
                nc.vector.tensor_copy(
                    xts[:, q * 512 : (q + 1) * 512], pt[:].bitcast(f32)
                )

            # ---- diagonal rows ----
            pt_d = ps_s.tile([16, 128], bf16, tag="ps_small")
            nc.tensor.transpose(pt_d[:], xdg[:, b * 16 : (b + 1) * 16], ident)
            xdgt = sm_p.tile([16, 128], bf16, tag="xdgt")
            nc.vector.tensor_copy(xdgt[:], pt_d[:])
            pzd = ps_s.tile([128, 32], f32, tag="ps_small")
            nc.tensor.matmul(pzd[:], lhsT=xdgt[:], rhs=w0d[:, 0:32], start=True, stop=False)
            nc.tensor.matmul(pzd[:], lhsT=rst, rhs=w0d[:, 32:64], start=False, stop=False)
            nc.tensor.matmul(pzd[:], lhsT=ones[:], rhs=tvs[:], start=False, stop=True)
            nc.vector.tensor_relu(zdall[:, b * 32 : (b + 1) * 32], pzd[:])

            # ---- main matmuls + relu + store ----
            osb = osb_p.tile([128, JF], f32, tag="osb")
            for s in range(8):
                po = ps_o.tile([128, 512], f32, tag="po")
                for h in range(2):
                    jb = 2 * s + h
                    ph = po[:, h * 256 : (h + 1) * 256]
                    nc.tensor.matmul(
                        ph,
                        lhsT=xts[:, jb * 128 : (jb + 1) * 128],
                        rhs=wblk,
                        start=True, stop=False,
                    )
                    nc.tensor.matmul(
                        ph, lhsT=rstcat[:],
                        rhs=w2cf[:, jb * 256 : (jb + 1) * 256],
                        start=False, stop=True,
                    )
                oslab = osb[:, s * 512 : (s + 1) * 512]
                if s % 2 == 0:
                    nc.scalar.activation(
                        out=oslab, in_=po[:],
                        func=mybir.ActivationFunctionType.Relu,
                    )
                else:
                    nc.vector.tensor_relu(oslab, po[:])
            # each half-store and its matching diagonal overwrite share one
            # HWDGE ring (SP for j<64, ACT for j>=64): per-SDMA-engine FIFO
            # order makes the overwrite land after the store with no
            # completion wait. Diag cell (i,i) for i<64 lies in half 0.
            o0 = out_h[:]
            half_dst = [
                bass.AP(
                    tensor=o0.tensor,
                    offset=o0.offset + b * N * JF + hh * (JF // 2),
                    ap=[[JF, 128], [1, JF // 2]],
                )
                for hh in range(2)
            ]
            diag_dst = [
                bass.AP(
                    tensor=o0.tensor,
                    offset=o0.offset + b * N * JF + hh * 64 * (N * F + F),
                    ap=[[N * F + F, 64], [1, F]],
                )
                for hh in range(2)
            ]
            for hh in range(2):
                eng = nc.scalar
                sth = eng.dma_start(
                    out=half_dst[hh],
                    in_=osb[:, hh * (JF // 2) : (hh + 1) * (JF // 2)],
                )
                dgh = eng.dma_start(
                    out=diag_dst[hh],
                    in_=zdall[hh * 64 : (hh + 1) * 64, b * 32 : (b + 1) * 32],
                )
                add_dep_helper(dgh.ins, sth.ins, sync=False,
                               reason="diag after store in ring order")

    nc.finalize()
    return nc


def _prep_consts(w, bias, diag_bias):
    w = np.asarray(w, np.float32)
    w0 = w[:, 0, :]
    w1s = w[:, 1, :] / NAVG**2
    w2s = w[:, 2, :] / NAVG
    w3s = w[:, 3, :] / NAVG
    w4s = w[:, 4, :] / NAVG
    w5s = w[:, 5, :] / NAVG**2
    wblk = np.zeros((128, 256), np.float32)
    for j8 in range(8):
        wblk[j8 * 16 : (j8 + 1) * 16, j8 * 32 : (j8 + 1) * 32] = w0
    import ml_dtypes

    bf16 = ml_dtypes.bfloat16
    CP = 256 + 128 + 64 + 64 + 64 + 64 + 4096
    cpack = np.zeros((128, CP), np.float32)
    cpack[:, 0:256] = wblk
    cpack[:, 256:384] = np.eye(128, dtype=np.float32)
    cpack[0:16, 384:448] = np.concatenate([w3s, w4s], 1)
    cpack[0:16, 448:512] = np.concatenate([w1s, w5s], 1)
    cpack[0:16, 512:576] = np.concatenate([w0, w2s + w3s + w4s], 1)
    cpack[0, 576:640] = np.concatenate(
        [np.asarray(bias, np.float32), np.asarray(diag_bias, np.float32)]
    )
    cpack[0:16, 640:4736] = np.tile(w2s, (1, 128))
    return {"cpack": cpack.astype(bf16),
            "identr": np.eye(128, dtype=np.float32)}


def _ensure_profile_hook():
    """Register the NTFF profile hook (the boot path skips it when the
    image lacks antenv.axon_hooks); needed only for trace=True runs."""
    import types

    try:
        from antenv.axon_hooks import get_axon_ntff_profile_hook  # noqa: F401
        return
    except ImportError:
        pass
    import antenv

    mod = types.ModuleType("antenv.axon_hooks")
    mod._hook = None
    mod.set_axon_ntff_profile_hook = lambda h: setattr(mod, "_hook", h)
    mod.get_axon_ntff_profile_hook = lambda: mod._hook
    sys.modules["antenv.axon_hooks"] = mod
    antenv.axon_hooks = mod
    try:
        from trn_agent_boot.trn_boot import _ntff_profile_via_ctypes

        mod._hook = _ntff_profile_via_ctypes("/opt/axon/libaxon_pjrt.so")
    except Exception as e:  # pragma: no cover
        print("profile hook setup failed:", e)


def kernel(inputs, w, bias, diag_bias):
    global LAST_EXEC_NS, LAST_RESULTS
    from concourse.bass_utils import run_bass_kernel_spmd

    if "nc" not in _CACHE:
        _CACHE["nc"] = _build_module()
    nc = _CACHE["nc"]

    x = np.ascontiguousarray(np.asarray(inputs, np.float32)).reshape(B, N, N * L)
    consts = _prep_consts(w, bias, diag_bias)

    in_maps = []
    for c in range(N_CORES):
        m = dict(consts)
        m["x"] = np.ascontiguousarray(x[c * B_LOC : (c + 1) * B_LOC])
        in_maps.append(m)

    trace = bool(int(os.environ.get("KERNEL_TRACE", "0")))
    if trace:
        _ensure_profile_hook()
    res = run_bass_kernel_spmd(nc, in_maps, list(range(N_CORES)), trace=trace)
    LAST_EXEC_NS = res.exec_time_ns
    LAST_RESULTS = res
    out = np.concatenate([res.results[c]["out"] for c in range(N_CORES)], axis=0)
    return out.reshape(B, N, N, F)

